# revision 13
# baseline (speedup 1.0000x reference)
"""Trainium2 Bass kernel for nn_GAT_88252987998923 (GNN message passing, 8 cores).

Math: with PASSES=1 the scatter-added h_prime feeds ONLY the mean readout
g = h_prime.mean(0).  Summing the per-edge scatter over all nodes and grouping
edges by destination, the segment-softmax attention weights sum to exactly 1
within each destination group, so

    g = (1/N) * sum_s (mask_s @ nodes) @ W[s],
    mask_s[n] = 1  iff  node n appears as a dst in edge set s,

and the attention parameters cancel entirely (verified vs the jax reference:
rel err ~2e-7).  The kernel computes mask_s on device via an indirect-DMA
scatter of 1.0s into a per-node table, reduces r_s[d] = sum_n mask_s[n]*h[n,d]
with accumulating PE matmuls, AllReduce-adds the [128,2] r across cores, and
evaluates the 3-layer MLP head on device.

Sharding: by destination-node range.  Core c owns nodes [c*12500,(c+1)*12500)
and every edge whose dst lands there, so masks are core-local (no cross-core
mask exchange; the only collective is the 1KB AllReduce of r).  The host only
slices/pads/permutes inputs; every reduction happens on device.
"""
import numpy as np

import concourse.bass as bass
import concourse.mybir as mybir
from concourse.bass_utils import run_bass_kernel_spmd
from concourse.masks import make_identity

NCORES = 8
N = 100000
D = 128
S = 2
NS = N // NCORES            # 12500 nodes per core
GRID_T = 99                 # free-dim node slots per partition
NSP = 128 * GRID_T          # 12672 padded nodes per core
PAD_IDX = 12600             # scatter target for padded edges (zero h row)
PADE = 76032                # padded edges per (core, set) = 594*128
NCHUNK = PADE // 128        # 594 indirect-DMA scatter instructions per set
NBUF = 8
HID = 80
OUT = 2
HEAD_IN = D + 1

_cache = {}


def _build():
    nc = bass.Bass(num_devices=NCORES)
    f32 = mybir.dt.float32

    h_in = nc.dram_tensor("h_pad", [NSP, D], f32, kind="ExternalInput")
    dst_in = nc.dram_tensor("dst_pad", [S, 128, NCHUNK], mybir.dt.int16,
                            kind="ExternalInput")
    w_in = nc.dram_tensor("W", [S, D, D], f32, kind="ExternalInput")
    pt_in = nc.dram_tensor("problem_type", [1, 1], f32, kind="ExternalInput")
    fc1w_in = nc.dram_tensor("fc1_w", [HID, HEAD_IN], f32, kind="ExternalInput")
    fc1b_in = nc.dram_tensor("fc1_b", [HID], f32, kind="ExternalInput")
    fc2w_in = nc.dram_tensor("fc2_w", [HID, HID], f32, kind="ExternalInput")
    fc2b_in = nc.dram_tensor("fc2_b", [HID], f32, kind="ExternalInput")
    fc3w_in = nc.dram_tensor("fc3_w", [OUT, HID], f32, kind="ExternalInput")
    fc3b_in = nc.dram_tensor("fc3_b", [OUT], f32, kind="ExternalInput")
    out_ext = nc.dram_tensor("out", [1, OUT], f32, kind="ExternalOutput")

    r_local = nc.dram_tensor("r_local", [D, S], f32)
    r_red = nc.dram_tensor("r_red", [D, S], f32, addr_space="Shared")


    with (
        nc.sbuf_tensor([128, GRID_T * D], f32) as h_sb,
        nc.sbuf_tensor([128, S * GRID_T], f32) as mask_sb,
        nc.sbuf_tensor([128, S * NCHUNK], mybir.dt.int32) as dst_sb,
        nc.sbuf_tensor([128, 2 * GRID_T], f32) as zeros_sb,
        nc.sbuf_tensor([128, 1], f32) as ones_sb,
        nc.sbuf_tensor([128, 128], f32) as ident,
        nc.sbuf_tensor([D, S], f32) as r_sb,
        nc.sbuf_tensor([D, S], f32) as rall_sb,
        nc.sbuf_tensor([128, S * D], f32) as w_sb,
        nc.sbuf_tensor([128, 1], f32) as g_sb,
        nc.sbuf_tensor([HID, HEAD_IN], f32) as f1_sb,
        nc.sbuf_tensor([HID, HID], f32) as f2_sb,
        nc.sbuf_tensor([OUT, HID], f32) as f3_sb,
        nc.sbuf_tensor([128, HID], f32) as f1t_sb,
        nc.sbuf_tensor([1, HID], f32) as f1tb_sb,
        nc.sbuf_tensor([HID, HID], f32) as f2t_sb,
        nc.sbuf_tensor([HID, OUT], f32) as f3t_sb,
        nc.sbuf_tensor([1, HID], f32) as b1_sb,
        nc.sbuf_tensor([1, HID], f32) as b2_sb,
        nc.sbuf_tensor([1, OUT], f32) as b3_sb,
        nc.sbuf_tensor([1, 1], f32) as pt_sb,
        nc.sbuf_tensor([1, HID], f32) as x1_sb,
        nc.sbuf_tensor([1, HID], f32) as x1m_sb,
        nc.sbuf_tensor([HID, 1], f32) as x1c_sb,
        nc.sbuf_tensor([1, HID], f32) as x2_sb,
        nc.sbuf_tensor([1, HID], f32) as x2m_sb,
        nc.sbuf_tensor([HID, 1], f32) as x2c_sb,
        nc.sbuf_tensor([1, OUT], f32) as o_sb,
        nc.psum_tensor([D, S], f32) as psum_r,
        nc.psum_tensor([D, 1], f32) as psum_g,
        nc.psum_tensor([128, HID], f32) as psum_t,
        nc.psum_tensor([1, HID], f32) as psum_x,
        nc.psum_tensor([HID, 1], f32) as psum_c,
        nc.psum_tensor([1, OUT], f32) as psum_o,
        nc.semaphore("s_w") as s_w,
        nc.semaphore("s_f") as s_f,
        nc.semaphore("s_h") as s_h,
        nc.semaphore("s_m") as s_m,
        nc.semaphore("s_z") as s_z,
        nc.semaphore("s_dst") as s_dst,
        nc.semaphore("s_sc") as s_sc,
        nc.semaphore("s_rl") as s_rl,
        nc.semaphore("s_ra") as s_ra,
        nc.semaphore("s_cc") as s_cc,
        nc.semaphore("s_id") as s_id,
        nc.semaphore("s_pe") as s_pe,
        nc.semaphore("s_ve") as s_ve,
        nc.semaphore("s_ac") as s_ac,
        nc.Block() as block,
    ):
        @block.sync
        def _(sy):
            for si in range(S):
                sy.dma_start(out=w_sb[:, si * D:(si + 1) * D],
                             in_=w_in[si]).then_inc(s_w, 16)
            sy.dma_start(out=f1_sb[:], in_=fc1w_in[:]).then_inc(s_f, 16)
            sy.dma_start(out=f2_sb[:], in_=fc2w_in[:]).then_inc(s_f, 16)
            sy.dma_start(out=f3_sb[:], in_=fc3w_in[:]).then_inc(s_f, 16)
            sy.dma_start(out=b1_sb[:], in_=fc1b_in[None, :]).then_inc(s_f, 16)
            sy.dma_start(out=b2_sb[:], in_=fc2b_in[None, :]).then_inc(s_f, 16)
            sy.dma_start(out=b3_sb[:], in_=fc3b_in[None, :]).then_inc(s_f, 16)
            sy.dma_start(out=pt_sb[:], in_=pt_in[:]).then_inc(s_f, 16)
            _hb = h_in[:]
            sy.dma_start(
                out=h_sb[:].rearrange("p (a d) -> p a d", d=D),
                in_=bass.AP(_hb.tensor, 0,
                            [[D, 128], [D * 128, GRID_T], [1, D]]),
            ).then_inc(s_h, 16)
            # final output store
            sy.wait_ge(s_ve, 10)
            sy.dma_start(out=out_ext[:], in_=o_sb[:]).then_inc(s_w, 16)

        @block.gpsimd
        def _(g):
            make_identity(nc, ident[:])
            g.iota(iota_b[:], pattern=[[1, 128]], base=0, channel_multiplier=0)
            g.iota(iota_a[:], pattern=[[1, GRID_T]], base=0,
                   channel_multiplier=0).then_inc(s_id, 1)
            for si in range(S):
                g.dma_start(
                    out=dst_sb[:, si * NCHUNK:(si + 1) * NCHUNK],
                    in_=dst_in[si],
                ).then_inc(s_dst, 16)
            g.wait_ge(s_ve, 5)                       # r_sb copied out of PSUM
            g.dma_start(out=r_local[:], in_=r_sb[:]).then_inc(s_rl, 16)
            g.wait_ge(s_rl, 16)
            g.collective_compute(
                "AllReduce",
                mybir.AluOpType.add,
                replica_groups=[list(range(NCORES))],
                ins=[r_local[:]],
                outs=[r_red[:]],
            ).then_inc(s_cc, 1)
            g.wait_ge(s_cc, 1)
            g.dma_start(out=rall_sb[:], in_=r_red[:]).then_inc(s_ra, 16)

        @block.tensor
        def _(t):
            # histogram: hist_s[b, a] += sum_e O[e,b] * A[e,a]
            for k in range(S * NCHUNK):
                t.wait_ge(s_hv, k + 1)
                slot = k % NBUF
                ps = psum_h0 if k < NCHUNK else psum_h1
                kk = k if k < NCHUNK else k - NCHUNK
                nc.tensor.matmul(
                    out=ps[:],
                    lhsT=obuf[:, slot * 128:(slot + 1) * 128],
                    rhs=abuf[:, slot * GRID_T:(slot + 1) * GRID_T],
                    start=(kk == 0),
                    stop=(kk == NCHUNK - 1),
                ).then_inc(s_hp, 1)
            # head-weight transposes (identity + f-loads first)
            t.wait_ge(s_id, 1)
            t.wait_ge(s_f, 112)
            nc.tensor.transpose(out=psum_t[:, :HID], in_=f1_sb[:, :128],
                                identity=ident[:HID, :HID]).then_inc(s_pe, 1)
            nc.tensor.transpose(out=psum_x[:], in_=f1_sb[:, 128:129],
                                identity=ident[:HID, :HID]).then_inc(s_pe, 1)
            t.wait_ge(s_ve, 2)
            nc.tensor.transpose(out=psum_t[:HID, :HID], in_=f2_sb[:],
                                identity=ident[:HID, :HID]).then_inc(s_pe, 1)
            t.wait_ge(s_ve, 3)
            nc.tensor.transpose(out=psum_t[:HID, :OUT], in_=f3_sb[:],
                                identity=ident[:OUT, :OUT]).then_inc(s_pe, 1)
            # r[d, s] = sum_n mask_s[n] h[n, d]
            t.wait_ge(s_h, 16)
            t.wait_ge(s_mk, 1)
            mm = None
            for ti in range(GRID_T):
                base = mask_sb[:, ti:ti + 1]
                rhs = bass.AP(base.tensor, base.offset,
                              [list(base.ap[0]), [GRID_T, S]])
                mm = nc.tensor.matmul(
                    out=psum_r[:],
                    lhsT=h_sb[:, ti * D:(ti + 1) * D],
                    rhs=rhs,
                    start=(ti == 0),
                    stop=(ti == GRID_T - 1),
                )
            mm.then_inc(s_pe, 1)                      # 5: psum_r ready
            # g = (r0 @ W0 + r1 @ W1)
            t.wait_ge(s_ra, 16)
            t.wait_ge(s_w, 32)
            for s in range(S):
                mm = nc.tensor.matmul(out=psum_g[:],
                                      lhsT=w_sb[:, s * D:(s + 1) * D],
                                      rhs=rall_sb[:, s:s + 1],
                                      start=(s == 0), stop=(s == S - 1))
            mm.then_inc(s_pe, 1)                      # 6: psum_g ready
            t.wait_ge(s_ac, 1)                        # g_sb scaled
            nc.tensor.matmul(out=psum_x[:], lhsT=g_sb[:], rhs=f1t_sb[:],
                             start=True, stop=False)
            nc.tensor.matmul(out=psum_x[:], lhsT=pt_sb[:], rhs=f1tb_sb[:],
                             start=False, stop=True).then_inc(s_pe, 1)     # 7
            t.wait_ge(s_ve, 6)                        # x1m ready
            nc.tensor.transpose(out=psum_c[:], in_=x1m_sb[:],
                                identity=ident[:1, :1]).then_inc(s_pe, 1)  # 8
            t.wait_ge(s_ve, 7)                        # x1c copied
            nc.tensor.matmul(out=psum_x[:], lhsT=x1c_sb[:], rhs=f2t_sb[:],
                             start=True, stop=True).then_inc(s_pe, 1)      # 9
            t.wait_ge(s_ve, 8)                        # x2m ready
            nc.tensor.transpose(out=psum_c[:], in_=x2m_sb[:],
                                identity=ident[:1, :1]).then_inc(s_pe, 1)  # 10
            t.wait_ge(s_ve, 9)                        # x2c copied
            nc.tensor.matmul(out=psum_o[:], lhsT=x2c_sb[:], rhs=f3t_sb[:],
                             start=True, stop=True).then_inc(s_pe, 1)      # 11

        @block.vector
        def _(v):
            v.wait_ge(s_id, 1)
            v.wait_ge(s_dst, 32)
            v.tensor_scalar(out=bmod_all[:], in0=dst_sb[:], scalar1=127,
                            scalar2=None, op0=mybir.AluOpType.bitwise_and)
            v.tensor_scalar(out=adiv_all[:], in0=dst_sb[:], scalar1=7,
                            scalar2=None,
                            op0=mybir.AluOpType.logical_shift_right)
            for k in range(S * NCHUNK):
                if k >= NBUF:
                    v.wait_ge(s_hp, k - NBUF + 1)
                slot = k % NBUF
                v.tensor_tensor(out=obuf[:, slot * 128:(slot + 1) * 128],
                                in0=bmod_all[:, k:k + 1].to_broadcast([128, 128]),
                                in1=iota_b[:], op=mybir.AluOpType.is_equal)
                v.tensor_tensor(out=abuf[:, slot * GRID_T:(slot + 1) * GRID_T],
                                in0=adiv_all[:, k:k + 1
                                             ].to_broadcast([128, GRID_T]),
                                in1=iota_a[:], op=mybir.AluOpType.is_equal
                                ).then_inc(s_hv, 1)
            # mask = hist > 0
            v.wait_ge(s_hp, S * NCHUNK)
            v.tensor_scalar(out=mask_sb[:, :GRID_T], in0=psum_h0[:],
                            scalar1=0, scalar2=None, op0=mybir.AluOpType.is_gt)
            v.tensor_scalar(out=mask_sb[:, GRID_T:], in0=psum_h1[:],
                            scalar1=0, scalar2=None,
                            op0=mybir.AluOpType.is_gt).then_inc(s_mk, 1)
            v.wait_ge(s_pe, 1)
            v.tensor_copy(out=f1t_sb[:], in_=psum_t[:, :HID]).then_inc(s_ve, 1)
            v.wait_ge(s_pe, 2)
            v.tensor_copy(out=f1tb_sb[:], in_=psum_x[:]).then_inc(s_ve, 1)
            v.wait_ge(s_pe, 3)
            v.tensor_copy(out=f2t_sb[:], in_=psum_t[:HID, :HID]).then_inc(s_ve, 1)
            v.wait_ge(s_pe, 4)
            v.tensor_copy(out=f3t_sb[:], in_=psum_t[:HID, :OUT]).then_inc(s_ve, 1)
            v.wait_ge(s_pe, 5)
            v.tensor_copy(out=r_sb[:], in_=psum_r[:]).then_inc(s_ve, 1)   # 5
            v.wait_ge(s_pe, 7)
            v.tensor_add(out=x1_sb[:], in0=psum_x[:], in1=b1_sb[:])
            v.tensor_scalar_mul(out=x1m_sb[:], in0=x1_sb[:], scalar1=0.01)
            v.tensor_tensor(out=x1m_sb[:], in0=x1_sb[:], in1=x1m_sb[:],
                            op=mybir.AluOpType.max).then_inc(s_ve, 1)     # 6
            v.wait_ge(s_pe, 8)
            v.tensor_copy(out=x1c_sb[:], in_=psum_c[:]).then_inc(s_ve, 1)  # 7
            v.wait_ge(s_pe, 9)
            v.tensor_add(out=x2_sb[:], in0=psum_x[:], in1=b2_sb[:])
            v.tensor_scalar_mul(out=x2m_sb[:], in0=x2_sb[:], scalar1=0.01)
            v.tensor_tensor(out=x2m_sb[:], in0=x2_sb[:], in1=x2m_sb[:],
                            op=mybir.AluOpType.max).then_inc(s_ve, 1)     # 8
            v.wait_ge(s_pe, 10)
            v.tensor_copy(out=x2c_sb[:], in_=psum_c[:]).then_inc(s_ve, 1)  # 9
            v.wait_ge(s_pe, 11)
            v.tensor_add(out=o_sb[:], in0=psum_o[:], in1=b3_sb[:]
                         ).then_inc(s_ve, 1)                               # 10

        @block.scalar
        def _(a):
            a.wait_ge(s_pe, 6)
            nc.scalar.mul(g_sb[:], psum_g[:], 1.0 / N).then_inc(s_ac, 1)

    return nc


def _shard(inputs):
    nodes = np.ascontiguousarray(np.asarray(inputs["nodes"], dtype=np.float32))
    edges = np.asarray(inputs["edges"])
    dst = np.asarray(edges[:, :, 1], dtype=np.int64)

    small = {k: np.ascontiguousarray(np.asarray(inputs[k], np.float32))
             for k in ["W", "problem_type", "fc1_w", "fc1_b", "fc2_w", "fc2_b",
                       "fc3_w", "fc3_b"]}
    per_core = []
    for c in range(NCORES):
        lo, hi = c * NS, (c + 1) * NS
        h_pad = np.zeros((NSP, D), np.float32)
        h_pad[:NS] = nodes[lo:hi]
        dst_pad = np.full((S, PADE), PAD_IDX, np.int16)
        for s in range(S):
            sel = dst[s][(dst[s] >= lo) & (dst[s] < hi)] - lo
            assert sel.size <= PADE, f"core {c} set {s}: {sel.size} edges"
            dst_pad[s, :sel.size] = sel.astype(np.int16)
        dstw = dst_pad.reshape(S, NCHUNK, 128).transpose(0, 2, 1)
        m = {"h_pad": h_pad, "dst_pad": np.ascontiguousarray(dstw)}
        m.update(small)
        per_core.append(m)
    return per_core


def kernel(trace=False, **inputs) -> np.ndarray:
    if "nc" not in _cache:
        _cache["nc"] = _build()
    nc = _cache["nc"]
    in_maps = _shard(inputs)
    res = run_bass_kernel_spmd(nc, in_maps, core_ids=list(range(NCORES)),
                               trace=trace)
    _cache["last_result"] = res
    return res.results[0]["out"]


# revision 14
# speedup vs baseline: 1.2927x; 1.2927x over previous
"""Trainium2 Bass kernel for nn_GAT_88252987998923 (GNN message passing, 8 cores).

Math: with PASSES=1 the scatter-added h_prime feeds ONLY the mean readout
g = h_prime.mean(0).  Summing the per-edge scatter over all nodes and grouping
edges by destination, the segment-softmax attention weights sum to exactly 1
within each destination group, so

    g = (1/N) * sum_s (mask_s @ nodes) @ W[s],
    mask_s[n] = 1  iff  node n appears as a dst in edge set s,

and the attention parameters cancel entirely (verified vs the jax reference:
rel err ~2e-7).  The kernel computes mask_s on device via an indirect-DMA
scatter of 1.0s into a per-node table, reduces r_s[d] = sum_n mask_s[n]*h[n,d]
with accumulating PE matmuls, AllReduce-adds the [128,2] r across cores, and
evaluates the 3-layer MLP head on device.

Sharding: by destination-node range.  Core c owns nodes [c*12500,(c+1)*12500)
and every edge whose dst lands there, so masks are core-local (no cross-core
mask exchange; the only collective is the 1KB AllReduce of r).  The host only
slices/pads/permutes inputs; every reduction happens on device.
"""
import numpy as np

import concourse.bass as bass
import concourse.mybir as mybir
from concourse.bass_utils import run_bass_kernel_spmd
from concourse.masks import make_identity

NCORES = 8
N = 100000
D = 128
S = 2
NS = N // NCORES            # 12500 nodes per core
GRID_T = 99                 # free-dim node slots per partition
NSP = 128 * GRID_T          # 12672 padded nodes per core
PAD_IDX = 12600             # scatter target for padded edges (zero h row)
PADE = 76032                # padded edges per (core, set) = 594*128
NCHUNK = PADE // 128        # 594 indirect-DMA scatter instructions per set
NBUF = 8
HID = 80
OUT = 2
HEAD_IN = D + 1

_cache = {}


def _build():
    nc = bass.Bass(num_devices=NCORES)
    f32 = mybir.dt.float32

    h_in = nc.dram_tensor("h_pad", [NSP, D], f32, kind="ExternalInput")
    dst_in = nc.dram_tensor("dst_pad", [S, 128, NCHUNK], mybir.dt.int16,
                            kind="ExternalInput")
    w_in = nc.dram_tensor("W", [S, D, D], f32, kind="ExternalInput")
    pt_in = nc.dram_tensor("problem_type", [1, 1], f32, kind="ExternalInput")
    fc1w_in = nc.dram_tensor("fc1_w", [HID, HEAD_IN], f32, kind="ExternalInput")
    fc1b_in = nc.dram_tensor("fc1_b", [HID], f32, kind="ExternalInput")
    fc2w_in = nc.dram_tensor("fc2_w", [HID, HID], f32, kind="ExternalInput")
    fc2b_in = nc.dram_tensor("fc2_b", [HID], f32, kind="ExternalInput")
    fc3w_in = nc.dram_tensor("fc3_w", [OUT, HID], f32, kind="ExternalInput")
    fc3b_in = nc.dram_tensor("fc3_b", [OUT], f32, kind="ExternalInput")
    out_ext = nc.dram_tensor("out", [1, OUT], f32, kind="ExternalOutput")

    r_local = nc.dram_tensor("r_local", [D, S], f32)
    r_red = nc.dram_tensor("r_red", [D, S], f32, addr_space="Shared")


    with (
        nc.sbuf_tensor([128, GRID_T * D], f32) as h_sb,
        nc.sbuf_tensor([128, S * GRID_T], f32) as mask_sb,
        nc.sbuf_tensor([128, S * NCHUNK], mybir.dt.int32) as dst_sb,
        nc.sbuf_tensor([128, 2 * GRID_T], f32) as zeros_sb,
        nc.sbuf_tensor([128, 1], f32) as ones_sb,
        nc.sbuf_tensor([128, 128], f32) as ident,
        nc.sbuf_tensor([D, S], f32) as r_sb,
        nc.sbuf_tensor([D, S], f32) as rall_sb,
        nc.sbuf_tensor([128, S * D], f32) as w_sb,
        nc.sbuf_tensor([128, 1], f32) as g_sb,
        nc.sbuf_tensor([HID, HEAD_IN], f32) as f1_sb,
        nc.sbuf_tensor([HID, HID], f32) as f2_sb,
        nc.sbuf_tensor([OUT, HID], f32) as f3_sb,
        nc.sbuf_tensor([128, HID], f32) as f1t_sb,
        nc.sbuf_tensor([1, HID], f32) as f1tb_sb,
        nc.sbuf_tensor([HID, HID], f32) as f2t_sb,
        nc.sbuf_tensor([HID, OUT], f32) as f3t_sb,
        nc.sbuf_tensor([1, HID], f32) as b1_sb,
        nc.sbuf_tensor([1, HID], f32) as b2_sb,
        nc.sbuf_tensor([1, OUT], f32) as b3_sb,
        nc.sbuf_tensor([1, 1], f32) as pt_sb,
        nc.sbuf_tensor([1, HID], f32) as x1_sb,
        nc.sbuf_tensor([1, HID], f32) as x1m_sb,
        nc.sbuf_tensor([HID, 1], f32) as x1c_sb,
        nc.sbuf_tensor([1, HID], f32) as x2_sb,
        nc.sbuf_tensor([1, HID], f32) as x2m_sb,
        nc.sbuf_tensor([HID, 1], f32) as x2c_sb,
        nc.sbuf_tensor([1, OUT], f32) as o_sb,
        nc.psum_tensor([D, S], f32) as psum_r,
        nc.psum_tensor([D, 1], f32) as psum_g,
        nc.psum_tensor([128, HID], f32) as psum_t,
        nc.psum_tensor([1, HID], f32) as psum_x,
        nc.psum_tensor([HID, 1], f32) as psum_c,
        nc.psum_tensor([1, OUT], f32) as psum_o,
        nc.semaphore("s_w") as s_w,
        nc.semaphore("s_f") as s_f,
        nc.semaphore("s_h") as s_h,
        nc.semaphore("s_m") as s_m,
        nc.semaphore("s_z") as s_z,
        nc.semaphore("s_dst") as s_dst,
        nc.semaphore("s_sc") as s_sc,
        nc.semaphore("s_rl") as s_rl,
        nc.semaphore("s_ra") as s_ra,
        nc.semaphore("s_cc") as s_cc,
        nc.semaphore("s_id") as s_id,
        nc.semaphore("s_pe") as s_pe,
        nc.semaphore("s_ve") as s_ve,
        nc.semaphore("s_ac") as s_ac,
        nc.Block() as block,
    ):
        @block.sync
        def _(sy):
            for si in range(S):
                sy.dma_start(out=w_sb[:, si * D:(si + 1) * D],
                             in_=w_in[si]).then_inc(s_w, 16)
            sy.dma_start(out=f1_sb[:], in_=fc1w_in[:]).then_inc(s_f, 16)
            sy.dma_start(out=f2_sb[:], in_=fc2w_in[:]).then_inc(s_f, 16)
            sy.dma_start(out=f3_sb[:], in_=fc3w_in[:]).then_inc(s_f, 16)
            sy.dma_start(out=b1_sb[:], in_=fc1b_in[None, :]).then_inc(s_f, 16)
            sy.dma_start(out=b2_sb[:], in_=fc2b_in[None, :]).then_inc(s_f, 16)
            sy.dma_start(out=b3_sb[:], in_=fc3b_in[None, :]).then_inc(s_f, 16)
            sy.dma_start(out=pt_sb[:], in_=pt_in[:]).then_inc(s_f, 16)
            _hb = h_in[:]
            sy.dma_start(
                out=h_sb[:].rearrange("p (a d) -> p a d", d=D),
                in_=bass.AP(_hb.tensor, 0,
                            [[D, 128], [D * 128, GRID_T], [1, D]]),
            ).then_inc(s_h, 16)
            # final output store
            sy.wait_ge(s_ve, 10)
            sy.dma_start(out=out_ext[:], in_=o_sb[:]).then_inc(s_w, 16)

        @block.gpsimd
        def _(g):
            make_identity(nc, ident[:])
            g.iota(iota_b[:], pattern=[[1, 128]], base=0, channel_multiplier=0)
            g.iota(iota_a[:], pattern=[[1, GRID_T]], base=0,
                   channel_multiplier=0).then_inc(s_id, 1)
            for si in range(S):
                g.dma_start(
                    out=dst_sb[:, si * NCHUNK:(si + 1) * NCHUNK],
                    in_=dst_in[si],
                ).then_inc(s_dst, 16)
            g.wait_ge(s_ve, 5)                       # r_sb copied out of PSUM
            g.dma_start(out=r_local[:], in_=r_sb[:]).then_inc(s_rl, 16)
            g.wait_ge(s_rl, 16)
            g.collective_compute(
                "AllReduce",
                mybir.AluOpType.add,
                replica_groups=[list(range(NCORES))],
                ins=[r_local[:]],
                outs=[r_red[:]],
            ).then_inc(s_cc, 1)
            g.wait_ge(s_cc, 1)
            g.dma_start(out=rall_sb[:], in_=r_red[:]).then_inc(s_ra, 16)

        @block.tensor
        def _(t):
            # histogram: hist_s[b, a] += sum_e O[e,b] * A[e,a]
            for k in range(S * NCHUNK):
                t.wait_ge(s_hv, k // 4 + 1)
                slot = k % NBUF
                ps = psum_h0 if k < NCHUNK else psum_h1
                kk = k if k < NCHUNK else k - NCHUNK
                nc.tensor.matmul(
                    out=ps[:],
                    lhsT=obuf[:, slot * 128:(slot + 1) * 128],
                    rhs=abuf[:, slot * GRID_T:(slot + 1) * GRID_T],
                    start=(kk == 0),
                    stop=(kk == NCHUNK - 1),
                ).then_inc(s_hp, 1)
            # head-weight transposes (identity + f-loads first)
            t.wait_ge(s_id, 1)
            t.wait_ge(s_f, 112)
            nc.tensor.transpose(out=psum_t[:, :HID], in_=f1_sb[:, :128],
                                identity=ident[:HID, :HID]).then_inc(s_pe, 1)
            nc.tensor.transpose(out=psum_x[:], in_=f1_sb[:, 128:129],
                                identity=ident[:HID, :HID]).then_inc(s_pe, 1)
            t.wait_ge(s_ve, 2)
            nc.tensor.transpose(out=psum_t[:HID, :HID], in_=f2_sb[:],
                                identity=ident[:HID, :HID]).then_inc(s_pe, 1)
            t.wait_ge(s_ve, 3)
            nc.tensor.transpose(out=psum_t[:HID, :OUT], in_=f3_sb[:],
                                identity=ident[:OUT, :OUT]).then_inc(s_pe, 1)
            # r[d, s] = sum_n mask_s[n] h[n, d]
            t.wait_ge(s_h, 16)
            t.wait_ge(s_mk, 1)
            mm = None
            for ti in range(GRID_T):
                base = mask_sb[:, ti:ti + 1]
                rhs = bass.AP(base.tensor, base.offset,
                              [list(base.ap[0]), [GRID_T, S]])
                mm = nc.tensor.matmul(
                    out=psum_r[:],
                    lhsT=h_sb[:, ti * D:(ti + 1) * D],
                    rhs=rhs,
                    start=(ti == 0),
                    stop=(ti == GRID_T - 1),
                )
            mm.then_inc(s_pe, 1)                      # 5: psum_r ready
            # g = (r0 @ W0 + r1 @ W1)
            t.wait_ge(s_ra, 16)
            t.wait_ge(s_w, 32)
            for s in range(S):
                mm = nc.tensor.matmul(out=psum_g[:],
                                      lhsT=w_sb[:, s * D:(s + 1) * D],
                                      rhs=rall_sb[:, s:s + 1],
                                      start=(s == 0), stop=(s == S - 1))
            mm.then_inc(s_pe, 1)                      # 6: psum_g ready
            t.wait_ge(s_ac, 1)                        # g_sb scaled
            nc.tensor.matmul(out=psum_x[:], lhsT=g_sb[:], rhs=f1t_sb[:],
                             start=True, stop=False)
            nc.tensor.matmul(out=psum_x[:], lhsT=pt_sb[:], rhs=f1tb_sb[:],
                             start=False, stop=True).then_inc(s_pe, 1)     # 7
            t.wait_ge(s_ve, 6)                        # x1m ready
            nc.tensor.transpose(out=psum_c[:], in_=x1m_sb[:],
                                identity=ident[:1, :1]).then_inc(s_pe, 1)  # 8
            t.wait_ge(s_ve, 7)                        # x1c copied
            nc.tensor.matmul(out=psum_x[:], lhsT=x1c_sb[:], rhs=f2t_sb[:],
                             start=True, stop=True).then_inc(s_pe, 1)      # 9
            t.wait_ge(s_ve, 8)                        # x2m ready
            nc.tensor.transpose(out=psum_c[:], in_=x2m_sb[:],
                                identity=ident[:1, :1]).then_inc(s_pe, 1)  # 10
            t.wait_ge(s_ve, 9)                        # x2c copied
            nc.tensor.matmul(out=psum_o[:], lhsT=x2c_sb[:], rhs=f3t_sb[:],
                             start=True, stop=True).then_inc(s_pe, 1)      # 11

        @block.vector
        def _(v):
            v.wait_ge(s_id, 1)
            v.wait_ge(s_dst, 32)
            v.tensor_scalar(out=bmod_all[:], in0=dst_sb[:], scalar1=127,
                            scalar2=None, op0=mybir.AluOpType.bitwise_and)
            v.tensor_scalar(out=adiv_all[:], in0=dst_sb[:], scalar1=7,
                            scalar2=None,
                            op0=mybir.AluOpType.logical_shift_right)
            def _b3(ap2, reps, inner):
                # [128, F] 2D slice -> [128, F, inner] with step-0 inner bcast
                return bass.AP(ap2.tensor, ap2.offset,
                               [list(ap2.ap[0]), [1, reps], [0, inner]])

            def _i3(ap2, reps, inner):
                # [128, inner] tile -> [128, reps, inner], step-0 reps bcast
                return bass.AP(ap2.tensor, ap2.offset,
                               [list(ap2.ap[0]), [0, reps], [1, inner]])

            for m in range(S * NCHUNK // 4):
                k0 = 4 * m
                if k0 >= NBUF:
                    v.wait_ge(s_hp, k0 - NBUF + 1)
                gs = (m % 2) * 4          # slot group: 0..3 or 4..7
                v.tensor_tensor(
                    out=obuf[:, gs * 128:(gs + 4) * 128
                             ].rearrange("p (f d) -> p f d", d=128),
                    in0=_b3(bmod_all[:, k0:k0 + 4], 4, 128),
                    in1=_i3(iota_b[:], 4, 128),
                    op=mybir.AluOpType.is_equal)
                v.tensor_tensor(
                    out=abuf[:, gs * GRID_T:(gs + 4) * GRID_T
                             ].rearrange("p (f d) -> p f d", d=GRID_T),
                    in0=_b3(adiv_all[:, k0:k0 + 4], 4, GRID_T),
                    in1=_i3(iota_a[:], 4, GRID_T),
                    op=mybir.AluOpType.is_equal).then_inc(s_hv, 1)
            # mask = hist > 0
            v.wait_ge(s_hp, S * NCHUNK)
            v.tensor_scalar(out=mask_sb[:, :GRID_T], in0=psum_h0[:],
                            scalar1=0, scalar2=None, op0=mybir.AluOpType.is_gt)
            v.tensor_scalar(out=mask_sb[:, GRID_T:], in0=psum_h1[:],
                            scalar1=0, scalar2=None,
                            op0=mybir.AluOpType.is_gt).then_inc(s_mk, 1)
            v.wait_ge(s_pe, 1)
            v.tensor_copy(out=f1t_sb[:], in_=psum_t[:, :HID]).then_inc(s_ve, 1)
            v.wait_ge(s_pe, 2)
            v.tensor_copy(out=f1tb_sb[:], in_=psum_x[:]).then_inc(s_ve, 1)
            v.wait_ge(s_pe, 3)
            v.tensor_copy(out=f2t_sb[:], in_=psum_t[:HID, :HID]).then_inc(s_ve, 1)
            v.wait_ge(s_pe, 4)
            v.tensor_copy(out=f3t_sb[:], in_=psum_t[:HID, :OUT]).then_inc(s_ve, 1)
            v.wait_ge(s_pe, 5)
            v.tensor_copy(out=r_sb[:], in_=psum_r[:]).then_inc(s_ve, 1)   # 5
            v.wait_ge(s_pe, 7)
            v.tensor_add(out=x1_sb[:], in0=psum_x[:], in1=b1_sb[:])
            v.tensor_scalar_mul(out=x1m_sb[:], in0=x1_sb[:], scalar1=0.01)
            v.tensor_tensor(out=x1m_sb[:], in0=x1_sb[:], in1=x1m_sb[:],
                            op=mybir.AluOpType.max).then_inc(s_ve, 1)     # 6
            v.wait_ge(s_pe, 8)
            v.tensor_copy(out=x1c_sb[:], in_=psum_c[:]).then_inc(s_ve, 1)  # 7
            v.wait_ge(s_pe, 9)
            v.tensor_add(out=x2_sb[:], in0=psum_x[:], in1=b2_sb[:])
            v.tensor_scalar_mul(out=x2m_sb[:], in0=x2_sb[:], scalar1=0.01)
            v.tensor_tensor(out=x2m_sb[:], in0=x2_sb[:], in1=x2m_sb[:],
                            op=mybir.AluOpType.max).then_inc(s_ve, 1)     # 8
            v.wait_ge(s_pe, 10)
            v.tensor_copy(out=x2c_sb[:], in_=psum_c[:]).then_inc(s_ve, 1)  # 9
            v.wait_ge(s_pe, 11)
            v.tensor_add(out=o_sb[:], in0=psum_o[:], in1=b3_sb[:]
                         ).then_inc(s_ve, 1)                               # 10

        @block.scalar
        def _(a):
            a.wait_ge(s_pe, 6)
            nc.scalar.mul(g_sb[:], psum_g[:], 1.0 / N).then_inc(s_ac, 1)

    return nc


def _shard(inputs):
    nodes = np.ascontiguousarray(np.asarray(inputs["nodes"], dtype=np.float32))
    edges = np.asarray(inputs["edges"])
    dst = np.asarray(edges[:, :, 1], dtype=np.int64)

    small = {k: np.ascontiguousarray(np.asarray(inputs[k], np.float32))
             for k in ["W", "problem_type", "fc1_w", "fc1_b", "fc2_w", "fc2_b",
                       "fc3_w", "fc3_b"]}
    per_core = []
    for c in range(NCORES):
        lo, hi = c * NS, (c + 1) * NS
        h_pad = np.zeros((NSP, D), np.float32)
        h_pad[:NS] = nodes[lo:hi]
        dst_pad = np.full((S, PADE), PAD_IDX, np.int16)
        for s in range(S):
            sel = dst[s][(dst[s] >= lo) & (dst[s] < hi)] - lo
            assert sel.size <= PADE, f"core {c} set {s}: {sel.size} edges"
            dst_pad[s, :sel.size] = sel.astype(np.int16)
        dstw = dst_pad.reshape(S, NCHUNK, 128).transpose(0, 2, 1)
        m = {"h_pad": h_pad, "dst_pad": np.ascontiguousarray(dstw)}
        m.update(small)
        per_core.append(m)
    return per_core


def kernel(trace=False, **inputs) -> np.ndarray:
    if "nc" not in _cache:
        _cache["nc"] = _build()
    nc = _cache["nc"]
    in_maps = _shard(inputs)
    res = run_bass_kernel_spmd(nc, in_maps, core_ids=list(range(NCORES)),
                               trace=trace)
    _cache["last_result"] = res
    return res.results[0]["out"]
